# revision 2
# baseline (speedup 1.0000x reference)
"""GQA attention block (rmsnorm + qk-headnorm + rotary + softmax + out-proj)
for Trainium2, SPMD over 8 NeuronCores: 2-way data parallel (batch) x 4-way
tensor parallel (query-head groups). Partial out-proj results are summed on
host (the TP all-reduce).

Shapes (hardcoded): tokens [2,2048,2048] f32, rotary [2048,64], norm_w [2048],
Wq [2048,2048], Wkv [2048,1024], Wo [2048,2048], gamma_q [32,64], gamma_k [8,64].

Per-core layout: pairs of heads packed on 128 partitions (2 x 64d). Attention
runs transposed (S^T = K^T-chunk @ Q^T strip), V is ones-augmented so the
softmax denominator falls out of the AV matmul's 65th row. Emission interleaves
pair p's attention (ACT-bound exp) with pair p+1's projections (PE-bound).
"""

import sys

for _p in ("/opt/trn_rl_repo", "/opt/pypackages"):
    if _p not in sys.path:
        sys.path.append(_p)

from contextlib import ExitStack

import numpy as np

import concourse.bass as bass
import concourse.tile as tile
from concourse import bacc, mybir
from concourse.bass_utils import run_bass_kernel_spmd

B, N, DIM = 2, 2048, 2048
DH = 64  # head dim
QH, KVH = 32, 8
NCORES = 8
TPG = 4  # tensor-parallel groups
QH_PER = QH // TPG  # 8 q heads per core
NPAIR = QH_PER // 2  # 4 pairs of q heads packed 2-per-128-partitions
NT = N // 128  # 16 token tiles
NC = DIM // 128  # 16 contraction chunks
STRIP = 512  # q-strip width in attention
NSTRIP = N // STRIP

F32 = mybir.dt.float32
F32R = mybir.dt.float32r
F16 = mybir.dt.float16

_BUILD = {}


def _build_nc():
    """Trace + compile the per-core Bass kernel (same program all cores)."""
    nc = bacc.Bacc(
        "TRN2", target_bir_lowering=False, debug=False, num_devices=NCORES
    )

    t_tokens = nc.dram_tensor("tokens", [N, DIM], F32, kind="ExternalInput").ap()
    t_wq = nc.dram_tensor("wq", [DIM, QH_PER * DH], F16, kind="ExternalInput").ap()
    t_wk = nc.dram_tensor("wk", [DIM, 2 * DH], F16, kind="ExternalInput").ap()
    t_wv = nc.dram_tensor("wv", [DIM, 2 * DH], F16, kind="ExternalInput").ap()
    t_wo = nc.dram_tensor("wo", [QH_PER * DH, DIM], F16, kind="ExternalInput").ap()
    t_gq = nc.dram_tensor("gq", [NPAIR * 128, 1], F32, kind="ExternalInput").ap()
    t_gk = nc.dram_tensor("gk", [128, 1], F32, kind="ExternalInput").ap()
    t_cos = nc.dram_tensor("cos_t", [128, N], F16, kind="ExternalInput").ap()
    t_sin = nc.dram_tensor("sins_t", [128, N], F16, kind="ExternalInput").ap()
    t_sel2 = nc.dram_tensor("sel2", [128, 2], F16, kind="ExternalInput").ap()
    t_bc2 = nc.dram_tensor("bc2", [2, 128], F32R, kind="ExternalInput").ap()
    t_ones = nc.dram_tensor("ones_col", [1, DH], F32R, kind="ExternalInput").ap()
    t_ident = nc.dram_tensor("ident", [128, 128], F16, kind="ExternalInput").ap()
    t_out = nc.dram_tensor("out", [N, DIM], F32, kind="ExternalOutput").ap()

    with tile.TileContext(nc) as tc, ExitStack() as ctx:
        # ---------------- persistent pools (whole kernel) ----------------
        persist = ctx.enter_context(tc.tile_pool(name="persist", bufs=1))
        qr_pool = ctx.enter_context(tc.tile_pool(name="qrp", bufs=1))

        # small constants
        eps_t = persist.tile([128, 1], F32)
        nc.vector.memset(eps_t, float(np.finfo(np.float32).eps))
        tiny_t = persist.tile([128, 1], F32)
        nc.vector.memset(tiny_t, 1e-24)
        zero_t = persist.tile([128, 1], F32)
        nc.vector.memset(zero_t, 0.0)
        sel2_sb = persist.tile([128, 2], F16)
        nc.sync.dma_start(out=sel2_sb[:], in_=t_sel2)
        bc2_sb = persist.tile([2, 128], F32R)
        nc.sync.dma_start(out=bc2_sb[:], in_=t_bc2)
        ones_sb = persist.tile([1, DH], F32R)
        nc.sync.dma_start(out=ones_sb[:], in_=t_ones)
        ident_sb = persist.tile([128, 128], F16)
        nc.sync.dma_start(out=ident_sb[:], in_=t_ident)
        gq_sb = persist.tile([128, NPAIR], F32)
        nc.sync.dma_start(
            out=gq_sb[:], in_=t_gq.rearrange("(p o) u -> o (p u)", o=128)
        )
        gk_sb = persist.tile([128, 1], F32)
        nc.sync.dma_start(out=gk_sb[:], in_=t_gk)

        # rotated q/k in f16, [2 heads * 64d, ntok]
        qr_t = [
            qr_pool.tile([128, N], F16, tag=f"qr{p}", name=f"qr{p}")
            for p in range(NPAIR)
        ]
        kr_t = qr_pool.tile([128, N], F16, tag="kr")
        # augmented V (natural layout): [tok-part, chunk, [v0|1|v1|1]]
        v_all = qr_pool.tile([128, NT, 2 * DH + 2], F16, tag="vall")
        nc.vector.memset(v_all[:, :, DH : DH + 1], 1.0)
        nc.vector.memset(v_all[:, :, 2 * DH + 1 : 2 * DH + 2], 1.0)
        # normalized attention outputs per pair [2 heads * 64d, ntok]
        o_t = [
            qr_pool.tile([128, N], F16, tag=f"o{p}", name=f"o{p}")
            for p in range(NPAIR)
        ]

        with ExitStack() as abc:
            xt_pool = abc.enter_context(tc.tile_pool(name="xtp", bufs=1))
            # x^T, f16, per 512-token strip: [dim-in-chunk-part, chunk, 512]
            xt_s = [
                xt_pool.tile([128, NC, 512], F16, tag=f"xt{s}", name=f"xt{s}")
                for s in range(4)
            ]

            with ExitStack() as bb:
                apool = bb.enter_context(tc.tile_pool(name="apool", bufs=2))
                spool = bb.enter_context(tc.tile_pool(name="spool", bufs=3))
                wpool = bb.enter_context(tc.tile_pool(name="wpool", bufs=1))
                cpool = bb.enter_context(tc.tile_pool(name="cpool", bufs=1))
                hpool = bb.enter_context(tc.tile_pool(name="hpool", bufs=2))
                epool = bb.enter_context(tc.tile_pool(name="epool", bufs=3))
                dpool = bb.enter_context(tc.tile_pool(name="dpool", bufs=2))
                # PSUM budget (8 banks): sps 2x2 (S-tiles + q-proj acc)
                # + ops 2 (o-acc, v-acc, transposes) + bc 2 (hsum/bcast/rps
                # + transposes)
                bcast_pool = bb.enter_context(
                    tc.tile_pool(name="bcp", bufs=2, space="PSUM")
                )
                drpool = bb.enter_context(
                    tc.tile_pool(name="drp", bufs=4, space="DRAM")
                )
                sps_pool = bb.enter_context(
                    tc.tile_pool(name="sps", bufs=2, space="PSUM")
                )
                ops_pool = bb.enter_context(
                    tc.tile_pool(name="ops", bufs=1, space="PSUM")
                )

                cos_sb = wpool.tile([128, N], F16)
                nc.gpsimd.dma_start(out=cos_sb[:], in_=t_cos)
                sin_sb = wpool.tile([128, N], F16)
                nc.gpsimd.dma_start(out=sin_sb[:], in_=t_sin)
                wq_sb = wpool.tile([128, NC, QH_PER * DH], F16)
                nc.gpsimd.dma_start(
                    out=wq_sb[:], in_=t_wq.rearrange("(c p) q -> p c q", p=128)
                )
                wk_sb = wpool.tile([128, NC, 2 * DH], F16)
                nc.gpsimd.dma_start(
                    out=wk_sb[:], in_=t_wk.rearrange("(c p) q -> p c q", p=128)
                )
                wv_sb = wpool.tile([128, NC, 2 * DH], F16)
                nc.gpsimd.dma_start(
                    out=wv_sb[:], in_=t_wv.rearrange("(c p) q -> p c q", p=128)
                )

                def a_tile(tt):
                    """rmsnorm + transpose token-tile tt into xt_s."""
                    x_nat = apool.tile([128, DIM], F32, tag="xnat", bufs=3)
                    nc.sync.dma_start(
                        out=x_nat[:], in_=t_tokens[tt * 128 : (tt + 1) * 128, :]
                    )
                    xs = apool.tile([128, DIM], F16, tag="xs")
                    ssum = spool.tile([128, 1], F32, tag="ssum")
                    nc.scalar.activation(
                        out=xs[:],
                        in_=x_nat[:],
                        func=mybir.ActivationFunctionType.Square,
                        bias=zero_t[:],
                        accum_out=ssum[:],
                    )
                    rstd = spool.tile([128, 1], F32, tag="rstd")
                    nc.scalar.activation(
                        out=rstd[:],
                        in_=ssum[:],
                        func=mybir.ActivationFunctionType.Sqrt,
                        bias=eps_t[:],
                        scale=1.0 / DIM,
                    )
                    nc.vector.reciprocal(out=rstd[:], in_=rstd[:])
                    nc.vector.tensor_scalar_mul(xs[:], x_nat[:], rstd[:])
                    st, off = tt // 4, (tt % 4) * 128
                    for cg in range(4):
                        if cg % 2 == 0:
                            tp = ops_pool.tile(
                                [128, 4, 128], F16, tag="ohi", name="tp"
                            )
                        else:
                            tp = bcast_pool.tile(
                                [128, 4, 128], F16, tag="bc", name="tp"
                            )
                        for j in range(4):
                            c = 4 * cg + j
                            nc.tensor.transpose(
                                tp[:, j, :],
                                xs[:, c * 128 : (c + 1) * 128],
                                ident_sb[:],
                            )
                        nc.vector.tensor_copy(
                            xt_s[st][:, 4 * cg : 4 * cg + 4, off : off + 128],
                            tp[:],
                        )

                def proj_norm_slice(proj_w, qn, s, gscale_ap):
                    """Project slice s, head-l2norm + gamma into qn[:, s]."""
                    ssl = slice(s * 512, (s + 1) * 512)
                    qps = sps_pool.tile([128, 512], F32, tag="sps", name="qps")
                    for c in range(NC):
                        nc.tensor.matmul(
                            qps[:],
                            proj_w(c),
                            xt_s[s][:, c, :],
                            start=(c == 0),
                            stop=(c == NC - 1),
                        )
                    sq2 = hpool.tile([128, 512], F16, tag="sq2")
                    nc.scalar.activation(
                        out=sq2[:],
                        in_=qps[:],
                        func=mybir.ActivationFunctionType.Square,
                        bias=zero_t[:],
                    )
                    hsum = bcast_pool.tile([2, 512], F32, tag="bc", name="hsum")
                    nc.tensor.matmul(
                        hsum[:], sel2_sb[:], sq2[:], start=True, stop=True
                    )
                    hrec = hpool.tile([2, 512], F32R, tag="hrec")
                    nc.scalar.activation(
                        out=hrec[:],
                        in_=hsum[:],
                        func=mybir.ActivationFunctionType.Sqrt,
                        bias=tiny_t[0:2],
                    )
                    with nc.allow_low_precision(reason="f32r rstd"):
                        nc.vector.reciprocal(out=hrec[:], in_=hrec[:])
                    bps = bcast_pool.tile([128, 512], F32, tag="bc")
                    nc.tensor.matmul(
                        bps[:], bc2_sb[:], hrec[:], start=True, stop=True
                    )
                    rsb = hpool.tile([128, 512], F16, tag="rsb")
                    nc.vector.tensor_scalar_mul(rsb[:], bps[:], gscale_ap)
                    nc.vector.tensor_mul(qn[:, ssl], qps[:], rsb[:])

                def rotary_finish(qn, dst):
                    # rotate-half swap within each head (partition +-32)
                    qsw = cpool.tile([128, N], F16, tag="qsw", bufs=2)
                    for h0 in (0, 64):
                        nc.sync.dma_start(
                            out=qsw[h0 : h0 + 32, :], in_=qn[h0 + 32 : h0 + 64, :]
                        )
                        nc.sync.dma_start(
                            out=qsw[h0 + 32 : h0 + 64, :], in_=qn[h0 : h0 + 32, :]
                        )
                    nc.vector.tensor_mul(qn[:], qn[:], cos_sb[:])
                    nc.vector.tensor_mul(qsw[:], qsw[:], sin_sb[:])
                    nc.vector.tensor_add(dst[:], qn[:], qsw[:])

                def headnorm_rotary(proj_w, dst, gscale_ap):
                    qn = cpool.tile([128, N], F16, tag="qn", bufs=2, name="qn")
                    for s in range(4):
                        proj_norm_slice(proj_w, qn, s, gscale_ap)
                    rotary_finish(qn, dst)

                def v_tile(tb):
                    vps = ops_pool.tile([128, 2 * DH], F32, tag="olo", name="vps")
                    for c in range(NC):
                        nc.tensor.matmul(
                            vps[:],
                            xt_s[tb // 4][:, c, (tb % 4) * 128 : (tb % 4 + 1) * 128],
                            wv_sb[:, c, :],
                            start=(c == 0),
                            stop=(c == NC - 1),
                        )
                    nc.vector.tensor_copy(v_all[:, tb, 0:DH], vps[:, 0:DH])
                    nc.vector.tensor_copy(
                        v_all[:, tb, DH + 1 : 2 * DH + 1], vps[:, DH : 2 * DH]
                    )

                # ---- interleaved prologue: per strip, A-tiles then K/V/Q0
                qn_k = cpool.tile([128, N], F16, tag="qn", bufs=2, name="qn_k")
                qn_q0 = cpool.tile([128, N], F16, tag="qn", bufs=2, name="qn_q0")
                for s in range(4):
                    for tt in range(4 * s, 4 * s + 4):
                        a_tile(tt)
                    proj_norm_slice(lambda c: wk_sb[:, c, :], qn_k, s, gk_sb[:])
                    for tb in range(4 * s, 4 * s + 4):
                        v_tile(tb)
                    proj_norm_slice(
                        lambda c: wq_sb[:, c, 0:128], qn_q0, s, gq_sb[:, 0:1]
                    )
                rotary_finish(qn_k, kr_t)
                rotary_finish(qn_q0, qr_t[0])

                def attention(p):
                    for st in range(NSTRIP):
                        ssl = slice(st * STRIP, (st + 1) * STRIP)
                        o_lo = ops_pool.tile([DH + 1, STRIP], F32, tag="olo")
                        o_hi = ops_pool.tile([DH + 1, STRIP], F32, tag="ohi")

                        def s_step(c):
                            ck = slice(c * 128, (c + 1) * 128)
                            sps = sps_pool.tile(
                                [128, 2, STRIP], F32, tag="sps"
                            )
                            nc.tensor.matmul(
                                sps[:, 0, :],
                                kr_t[0:DH, ck],
                                qr_t[p][0:DH, ssl],
                                start=True,
                                stop=True,
                            )
                            nc.tensor.matmul(
                                sps[:, 1, :],
                                kr_t[DH:128, ck],
                                qr_t[p][DH:128, ssl],
                                start=True,
                                stop=True,
                            )
                            return sps

                        def exp_step(sps):
                            et = epool.tile([128, 2, STRIP], F16, tag="et")
                            nc.scalar.activation(
                                out=et[:],
                                in_=sps[:],
                                func=mybir.ActivationFunctionType.Exp,
                                bias=zero_t[:],
                                scale=float(DH) ** -0.5,
                            )
                            return et

                        def av_step(c, et):
                            nc.tensor.matmul(
                                o_lo[:],
                                v_all[:, c, 0 : DH + 1],
                                et[:, 0, :],
                                start=(c == 0),
                                stop=(c == NC - 1),
                            )
                            nc.tensor.matmul(
                                o_hi[:],
                                v_all[:, c, DH + 1 : 2 * DH + 2],
                                et[:, 1, :],
                                start=(c == 0),
                                stop=(c == NC - 1),
                            )

                        # S(c+1) ahead of AV(c): the PE queue never waits
                        # on exp(c)
                        et = exp_step(s_step(0))
                        for c in range(NC - 1):
                            sps = s_step(c + 1)
                            av_step(c, et)
                            et = exp_step(sps)
                        av_step(NC - 1, et)
                        # normalize rows 0..63 by row 64 (softmax sum);
                        # copy BOTH psum banks out first so they free before
                        # the recip/mul chain occupies the DVE queue
                        ocps = []
                        for ops in (o_lo, o_hi):
                            ocp = dpool.tile([DH + 1, STRIP], F16, tag="ocp")
                            nc.vector.tensor_copy(ocp[:], ops[:])
                            ocps.append(ocp)
                        for half in (0, 1):
                            ocp = ocps[half]
                            den = dpool.tile([1, STRIP], F32R, tag="den")
                            with nc.allow_low_precision(reason="f32r den"):
                                nc.vector.reciprocal(
                                    out=den[:], in_=ocp[DH : DH + 1, :]
                                )
                            rps = bcast_pool.tile([DH, STRIP], F32, tag="bc")
                            nc.tensor.matmul(
                                rps[:], ones_sb[:], den[:], start=True, stop=True
                            )
                            rsb2 = dpool.tile([DH, STRIP], F16, tag="rsb2")
                            nc.vector.tensor_copy(rsb2[:], rps[:])
                            nc.vector.tensor_mul(
                                o_t[p][half * DH : (half + 1) * DH, ssl],
                                ocp[0:DH, :],
                                rsb2[:],
                            )

                # pipeline: headnorm(p+1) emitted before attention(p)
                for p in range(NPAIR):
                    if p + 1 < NPAIR:
                        headnorm_rotary(
                            lambda c, p=p: wq_sb[:, c, (p + 1) * 128 : (p + 2) * 128],
                            qr_t[p + 1],
                            gq_sb[:, p + 1 : p + 2],
                        )
                    attention(p)

        # ---------------- phase E: out-projection ----------------
        with ExitStack() as ee:
            wo_pool = ee.enter_context(tc.tile_pool(name="wop", bufs=1))
            outp = ee.enter_context(tc.tile_pool(name="outp", bufs=2))
            xps_pool = ee.enter_context(
                tc.tile_pool(name="xps", bufs=2, space="PSUM")
            )
            wo_sb = wo_pool.tile([128, NPAIR, DIM], F16)
            nc.sync.dma_start(
                out=wo_sb[:], in_=t_wo.rearrange("(p o) d -> o p d", o=128)
            )
            for tb in range(NT):
                xps = xps_pool.tile([128, DIM], F32, tag="xps")
                for p in range(NPAIR):
                    for s in range(4):
                        nc.tensor.matmul(
                            xps[:, s * 512 : (s + 1) * 512],
                            o_t[p][:, tb * 128 : (tb + 1) * 128],
                            wo_sb[:, p, s * 512 : (s + 1) * 512],
                            start=(p == 0),
                            stop=(p == NPAIR - 1),
                        )
                osb = outp.tile([128, DIM], F32, tag="osb")
                nc.vector.tensor_copy(osb[:], xps[:])
                nc.sync.dma_start(
                    out=t_out[tb * 128 : (tb + 1) * 128, :], in_=osb[:]
                )

    nc.compile()
    return nc


def _core_inputs(core, tokens, rotary, norm_w, Wq, Wkv, Wo, gamma_q, gamma_k):
    b, g = core // TPG, core % TPG
    # pair order: pair p = (8g+p, 8g+4+p); lo half -> kv head 2g, hi -> 2g+1
    heads = []
    for p in range(NPAIR):
        heads += [QH_PER * g + p, QH_PER * g + NPAIR + p]
    cols = np.concatenate([np.arange(h * DH, (h + 1) * DH) for h in heads])

    nw = norm_w[:, None].astype(np.float32)
    wq = (Wq * nw)[:, cols].astype(np.float16)
    kcols = np.arange(2 * g * DH, (2 * g + 2) * DH)
    wk = (Wkv[:, : KVH * DH] * nw)[:, kcols].astype(np.float16)
    wv = (Wkv[:, KVH * DH :] * nw)[:, kcols].astype(np.float16)
    wo = Wo[cols, :].astype(np.float16)

    gq = np.empty((NPAIR * 128, 1), np.float32)
    for p in range(NPAIR):
        gq[p * 128 : p * 128 + DH, 0] = (gamma_q[heads[2 * p]] + 1.0) * DH**0.5
        gq[p * 128 + DH : (p + 1) * 128, 0] = (
            gamma_q[heads[2 * p + 1]] + 1.0
        ) * DH**0.5
    gk = np.empty((128, 1), np.float32)
    gk[:DH, 0] = (gamma_k[2 * g] + 1.0) * DH**0.5
    gk[DH:, 0] = (gamma_k[2 * g + 1] + 1.0) * DH**0.5

    cosT = np.cos(rotary).T.astype(np.float32)  # [64, N]
    sinT = np.sin(rotary).T.astype(np.float32)
    sinS = np.concatenate([-sinT[:32], sinT[32:]], axis=0)
    cos_t = np.tile(cosT, (2, 1)).astype(np.float16)
    sins_t = np.tile(sinS, (2, 1)).astype(np.float16)

    sel2 = np.zeros((128, 2), np.float16)
    sel2[:DH, 0] = 1
    sel2[DH:, 1] = 1
    bc2 = np.zeros((2, 128), np.float32)
    bc2[0, :DH] = 1
    bc2[1, DH:] = 1

    return {
        "tokens": np.ascontiguousarray(tokens[b]),
        "wq": np.ascontiguousarray(wq),
        "wk": np.ascontiguousarray(wk),
        "wv": np.ascontiguousarray(wv),
        "wo": np.ascontiguousarray(wo),
        "gq": gq,
        "gk": gk,
        "cos_t": cos_t,
        "sins_t": sins_t,
        "sel2": sel2,
        "bc2": bc2,
        "ones_col": np.ones((1, DH), np.float32),
        "ident": np.eye(128, dtype=np.float16),
    }


def kernel(tokens, rotary, norm_w, Wq, Wkv, Wo, gamma_q, gamma_k, _bench=None):
    if "nc" not in _BUILD:
        _BUILD["nc"] = _build_nc()
    nc = _BUILD["nc"]

    in_maps = [
        _core_inputs(c, tokens, rotary, norm_w, Wq, Wkv, Wo, gamma_q, gamma_k)
        for c in range(NCORES)
    ]
    kw = dict(_bench or {})
    res = run_bass_kernel_spmd(nc, in_maps, list(range(NCORES)), **kw)
    if _bench is not None:
        _BUILD["last"] = res

    out = np.empty((B, N, DIM), np.float32)
    for b in range(B):
        acc = res.results[b * TPG]["out"].astype(np.float32)
        for g in range(1, TPG):
            acc = acc + res.results[b * TPG + g]["out"]
        out[b] = acc
    return out



# revision 3
# speedup vs baseline: 1.0021x; 1.0021x over previous
"""GQA attention block (rmsnorm + qk-headnorm + rotary + softmax + out-proj)
for Trainium2, SPMD over 8 NeuronCores: 2-way data parallel (batch) x 4-way
tensor parallel (query-head groups). Partial out-proj results are summed on
host (the TP all-reduce).

Shapes (hardcoded): tokens [2,2048,2048] f32, rotary [2048,64], norm_w [2048],
Wq [2048,2048], Wkv [2048,1024], Wo [2048,2048], gamma_q [32,64], gamma_k [8,64].

Per-core layout: pairs of heads packed on 128 partitions (2 x 64d). Attention
runs transposed (S^T = K^T-chunk @ Q^T strip), V is ones-augmented so the
softmax denominator falls out of the AV matmul's 65th row. Emission interleaves
pair p's attention (ACT-bound exp) with pair p+1's projections (PE-bound).
"""

import sys

for _p in ("/opt/trn_rl_repo", "/opt/pypackages"):
    if _p not in sys.path:
        sys.path.append(_p)

from contextlib import ExitStack

import numpy as np

import concourse.bass as bass
import concourse.tile as tile
from concourse import bacc, mybir
from concourse.bass_utils import run_bass_kernel_spmd

B, N, DIM = 2, 2048, 2048
DH = 64  # head dim
QH, KVH = 32, 8
NCORES = 8
TPG = 4  # tensor-parallel groups
QH_PER = QH // TPG  # 8 q heads per core
NPAIR = QH_PER // 2  # 4 pairs of q heads packed 2-per-128-partitions
NT = N // 128  # 16 token tiles
NC = DIM // 128  # 16 contraction chunks
STRIP = 512  # q-strip width in attention
NSTRIP = N // STRIP

F32 = mybir.dt.float32
F32R = mybir.dt.float32r
F16 = mybir.dt.float16

_BUILD = {}


def _build_nc():
    """Trace + compile the per-core Bass kernel (same program all cores)."""
    nc = bacc.Bacc(
        "TRN2", target_bir_lowering=False, debug=False, num_devices=NCORES
    )

    t_tokens = nc.dram_tensor("tokens", [N, DIM], F32, kind="ExternalInput").ap()
    t_wq = nc.dram_tensor("wq", [DIM, QH_PER * DH], F16, kind="ExternalInput").ap()
    t_wk = nc.dram_tensor("wk", [DIM, 2 * DH], F16, kind="ExternalInput").ap()
    t_wv = nc.dram_tensor("wv", [DIM, 2 * DH], F16, kind="ExternalInput").ap()
    t_wo = nc.dram_tensor("wo", [QH_PER * DH, DIM], F16, kind="ExternalInput").ap()
    t_gq = nc.dram_tensor("gq", [NPAIR * 128, 1], F32, kind="ExternalInput").ap()
    t_gk = nc.dram_tensor("gk", [128, 1], F32, kind="ExternalInput").ap()
    t_cos = nc.dram_tensor("cos_t", [128, N], F16, kind="ExternalInput").ap()
    t_sin = nc.dram_tensor("sins_t", [128, N], F16, kind="ExternalInput").ap()
    t_sel2 = nc.dram_tensor("sel2", [128, 2], F16, kind="ExternalInput").ap()
    t_bc2 = nc.dram_tensor("bc2", [2, 128], F32R, kind="ExternalInput").ap()
    t_ones = nc.dram_tensor("ones_col", [1, DH], F32R, kind="ExternalInput").ap()
    t_ident = nc.dram_tensor("ident", [128, 128], F16, kind="ExternalInput").ap()
    t_out = nc.dram_tensor("out", [N, DIM], F32, kind="ExternalOutput").ap()

    with tile.TileContext(nc) as tc, ExitStack() as ctx:
        # ---------------- persistent pools (whole kernel) ----------------
        persist = ctx.enter_context(tc.tile_pool(name="persist", bufs=1))
        qr_pool = ctx.enter_context(tc.tile_pool(name="qrp", bufs=1))

        # small constants
        eps_t = persist.tile([128, 1], F32)
        nc.vector.memset(eps_t, float(np.finfo(np.float32).eps))
        tiny_t = persist.tile([128, 1], F32)
        nc.vector.memset(tiny_t, 1e-24)
        zero_t = persist.tile([128, 1], F32)
        nc.vector.memset(zero_t, 0.0)
        sel2_sb = persist.tile([128, 2], F16)
        nc.sync.dma_start(out=sel2_sb[:], in_=t_sel2)
        bc2_sb = persist.tile([2, 128], F32R)
        nc.sync.dma_start(out=bc2_sb[:], in_=t_bc2)
        ones_sb = persist.tile([1, DH], F32R)
        nc.sync.dma_start(out=ones_sb[:], in_=t_ones)
        ident_sb = persist.tile([128, 128], F16)
        nc.sync.dma_start(out=ident_sb[:], in_=t_ident)
        gq_sb = persist.tile([128, NPAIR], F32)
        nc.sync.dma_start(
            out=gq_sb[:], in_=t_gq.rearrange("(p o) u -> o (p u)", o=128)
        )
        gk_sb = persist.tile([128, 1], F32)
        nc.sync.dma_start(out=gk_sb[:], in_=t_gk)

        # rotated q/k in f16, [2 heads * 64d, ntok]
        qr_t = [
            qr_pool.tile([128, N], F16, tag=f"qr{p}", name=f"qr{p}")
            for p in range(NPAIR)
        ]
        kr_t = qr_pool.tile([128, N], F16, tag="kr")
        # augmented V (natural layout): [tok-part, chunk, [v0|1|v1|1]]
        v_all = qr_pool.tile([128, NT, 2 * DH + 2], F16, tag="vall")
        nc.vector.memset(v_all[:, :, DH : DH + 1], 1.0)
        nc.vector.memset(v_all[:, :, 2 * DH + 1 : 2 * DH + 2], 1.0)
        # normalized attention outputs per pair [2 heads * 64d, ntok]
        o_t = [
            qr_pool.tile([128, N], F16, tag=f"o{p}", name=f"o{p}")
            for p in range(NPAIR)
        ]

        with ExitStack() as abc:
            xt_pool = abc.enter_context(tc.tile_pool(name="xtp", bufs=1))
            # x^T, f16, per 512-token strip: [dim-in-chunk-part, chunk, 512]
            xt_s = [
                xt_pool.tile([128, NC, 512], F16, tag=f"xt{s}", name=f"xt{s}")
                for s in range(4)
            ]

            with ExitStack() as bb:
                apool = bb.enter_context(tc.tile_pool(name="apool", bufs=2))
                spool = bb.enter_context(tc.tile_pool(name="spool", bufs=3))
                wpool = bb.enter_context(tc.tile_pool(name="wpool", bufs=1))
                cpool = bb.enter_context(tc.tile_pool(name="cpool", bufs=1))
                hpool = bb.enter_context(tc.tile_pool(name="hpool", bufs=2))
                epool = bb.enter_context(tc.tile_pool(name="epool", bufs=3))
                dpool = bb.enter_context(tc.tile_pool(name="dpool", bufs=2))
                # PSUM budget (8 banks): sps 2x2 (S-tiles + q-proj acc)
                # + ops 2 (o-acc, v-acc, transposes) + bc 2 (hsum/bcast/rps
                # + transposes)
                bcast_pool = bb.enter_context(
                    tc.tile_pool(name="bcp", bufs=2, space="PSUM")
                )
                drpool = bb.enter_context(
                    tc.tile_pool(name="drp", bufs=4, space="DRAM")
                )
                sps_pool = bb.enter_context(
                    tc.tile_pool(name="sps", bufs=2, space="PSUM")
                )
                ops_pool = bb.enter_context(
                    tc.tile_pool(name="ops", bufs=1, space="PSUM")
                )

                cos_sb = wpool.tile([128, N], F16)
                nc.gpsimd.dma_start(out=cos_sb[:], in_=t_cos)
                sin_sb = wpool.tile([128, N], F16)
                nc.gpsimd.dma_start(out=sin_sb[:], in_=t_sin)
                wq_sb = wpool.tile([128, NC, QH_PER * DH], F16)
                nc.gpsimd.dma_start(
                    out=wq_sb[:], in_=t_wq.rearrange("(c p) q -> p c q", p=128)
                )
                wk_sb = wpool.tile([128, NC, 2 * DH], F16)
                nc.gpsimd.dma_start(
                    out=wk_sb[:], in_=t_wk.rearrange("(c p) q -> p c q", p=128)
                )
                wv_sb = wpool.tile([128, NC, 2 * DH], F16)
                nc.gpsimd.dma_start(
                    out=wv_sb[:], in_=t_wv.rearrange("(c p) q -> p c q", p=128)
                )

                def a_tile(tt):
                    """rmsnorm + transpose token-tile tt into xt_s."""
                    x_nat = apool.tile([128, DIM], F32, tag="xnat", bufs=3)
                    nc.sync.dma_start(
                        out=x_nat[:], in_=t_tokens[tt * 128 : (tt + 1) * 128, :]
                    )
                    xs = apool.tile([128, DIM], F16, tag="xs")
                    ssum = spool.tile([128, 1], F32, tag="ssum")
                    nc.scalar.activation(
                        out=xs[:],
                        in_=x_nat[:],
                        func=mybir.ActivationFunctionType.Square,
                        bias=zero_t[:],
                        accum_out=ssum[:],
                    )
                    rstd = spool.tile([128, 1], F32, tag="rstd")
                    nc.scalar.activation(
                        out=rstd[:],
                        in_=ssum[:],
                        func=mybir.ActivationFunctionType.Sqrt,
                        bias=eps_t[:],
                        scale=1.0 / DIM,
                    )
                    nc.vector.reciprocal(out=rstd[:], in_=rstd[:])
                    nc.vector.tensor_scalar_mul(xs[:], x_nat[:], rstd[:])
                    st, off = tt // 4, (tt % 4) * 128
                    for cg in range(4):
                        if cg % 2 == 0:
                            tp = ops_pool.tile(
                                [128, 4, 128], F16, tag="ohi", name="tp"
                            )
                        else:
                            tp = bcast_pool.tile(
                                [128, 4, 128], F16, tag="bc", name="tp"
                            )
                        for j in range(4):
                            c = 4 * cg + j
                            nc.tensor.transpose(
                                tp[:, j, :],
                                xs[:, c * 128 : (c + 1) * 128],
                                ident_sb[:],
                            )
                        nc.vector.tensor_copy(
                            xt_s[st][:, 4 * cg : 4 * cg + 4, off : off + 128],
                            tp[:],
                        )

                def proj_norm_slice(proj_w, qn, s, gscale_ap):
                    """Project slice s, head-l2norm + gamma into qn[:, s]."""
                    ssl = slice(s * 512, (s + 1) * 512)
                    qps = sps_pool.tile([128, 512], F32, tag="sps", name="qps")
                    for c in range(NC):
                        nc.tensor.matmul(
                            qps[:],
                            proj_w(c),
                            xt_s[s][:, c, :],
                            start=(c == 0),
                            stop=(c == NC - 1),
                        )
                    sq2 = hpool.tile([128, 512], F16, tag="sq2")
                    nc.scalar.activation(
                        out=sq2[:],
                        in_=qps[:],
                        func=mybir.ActivationFunctionType.Square,
                        bias=zero_t[:],
                    )
                    hsum = bcast_pool.tile([2, 512], F32, tag="bc", name="hsum")
                    nc.tensor.matmul(
                        hsum[:], sel2_sb[:], sq2[:], start=True, stop=True
                    )
                    hrec = hpool.tile([2, 512], F32R, tag="hrec")
                    nc.scalar.activation(
                        out=hrec[:],
                        in_=hsum[:],
                        func=mybir.ActivationFunctionType.Sqrt,
                        bias=tiny_t[0:2],
                    )
                    with nc.allow_low_precision(reason="f32r rstd"):
                        nc.vector.reciprocal(out=hrec[:], in_=hrec[:])
                    bps = bcast_pool.tile([128, 512], F32, tag="bc")
                    nc.tensor.matmul(
                        bps[:], bc2_sb[:], hrec[:], start=True, stop=True
                    )
                    rsb = hpool.tile([128, 512], F16, tag="rsb")
                    nc.vector.tensor_scalar_mul(rsb[:], bps[:], gscale_ap)
                    nc.vector.tensor_mul(qn[:, ssl], qps[:], rsb[:])

                def rotary_finish(qn, dst):
                    # rotate-half swap within each head (partition +-32)
                    qsw = cpool.tile([128, N], F16, tag="qsw", bufs=2)
                    for h0 in (0, 64):
                        nc.sync.dma_start(
                            out=qsw[h0 : h0 + 32, :], in_=qn[h0 + 32 : h0 + 64, :]
                        )
                        nc.sync.dma_start(
                            out=qsw[h0 + 32 : h0 + 64, :], in_=qn[h0 : h0 + 32, :]
                        )
                    nc.vector.tensor_mul(qn[:], qn[:], cos_sb[:])
                    nc.vector.tensor_mul(qsw[:], qsw[:], sin_sb[:])
                    nc.vector.tensor_add(dst[:], qn[:], qsw[:])

                def headnorm_rotary(proj_w, dst, gscale_ap):
                    qn = cpool.tile([128, N], F16, tag="qn", bufs=2, name="qn")
                    for s in range(4):
                        proj_norm_slice(proj_w, qn, s, gscale_ap)
                    rotary_finish(qn, dst)

                def v_tile(tb):
                    vps = ops_pool.tile([128, 2 * DH], F32, tag="olo", name="vps")
                    for c in range(NC):
                        nc.tensor.matmul(
                            vps[:],
                            xt_s[tb // 4][:, c, (tb % 4) * 128 : (tb % 4 + 1) * 128],
                            wv_sb[:, c, :],
                            start=(c == 0),
                            stop=(c == NC - 1),
                        )
                    nc.vector.tensor_copy(v_all[:, tb, 0:DH], vps[:, 0:DH])
                    nc.vector.tensor_copy(
                        v_all[:, tb, DH + 1 : 2 * DH + 1], vps[:, DH : 2 * DH]
                    )

                # ---- interleaved prologue: per strip, A-tiles then K/V/Q0
                qn_k = cpool.tile([128, N], F16, tag="qn", bufs=2, name="qn_k")
                qn_q0 = cpool.tile([128, N], F16, tag="qn", bufs=2, name="qn_q0")
                for s in range(4):
                    for tt in range(4 * s, 4 * s + 4):
                        a_tile(tt)
                    proj_norm_slice(lambda c: wk_sb[:, c, :], qn_k, s, gk_sb[:])
                    for tb in range(4 * s, 4 * s + 4):
                        v_tile(tb)
                    proj_norm_slice(
                        lambda c: wq_sb[:, c, 0:128], qn_q0, s, gq_sb[:, 0:1]
                    )
                rotary_finish(qn_k, kr_t)
                rotary_finish(qn_q0, qr_t[0])

                def attention(p):
                    for st in range(NSTRIP):
                        ssl = slice(st * STRIP, (st + 1) * STRIP)
                        o_lo = ops_pool.tile([DH + 1, STRIP], F32, tag="olo")
                        o_hi = ops_pool.tile([DH + 1, STRIP], F32, tag="ohi")

                        def s_step(c):
                            ck = slice(c * 128, (c + 1) * 128)
                            sps = sps_pool.tile(
                                [128, 2, STRIP], F32, tag="sps"
                            )
                            nc.tensor.matmul(
                                sps[:, 0, :],
                                kr_t[0:DH, ck],
                                qr_t[p][0:DH, ssl],
                                start=True,
                                stop=True,
                            )
                            nc.tensor.matmul(
                                sps[:, 1, :],
                                kr_t[DH:128, ck],
                                qr_t[p][DH:128, ssl],
                                start=True,
                                stop=True,
                            )
                            return sps

                        def exp_step(sps):
                            et = epool.tile([128, 2, STRIP], F16, tag="et")
                            nc.scalar.activation(
                                out=et[:],
                                in_=sps[:],
                                func=mybir.ActivationFunctionType.Exp,
                                bias=zero_t[:],
                                scale=float(DH) ** -0.5,
                            )
                            return et

                        def av_step(c, et):
                            nc.tensor.matmul(
                                o_lo[:],
                                v_all[:, c, 0 : DH + 1],
                                et[:, 0, :],
                                start=(c == 0),
                                stop=(c == NC - 1),
                            )
                            nc.tensor.matmul(
                                o_hi[:],
                                v_all[:, c, DH + 1 : 2 * DH + 2],
                                et[:, 1, :],
                                start=(c == 0),
                                stop=(c == NC - 1),
                            )

                        # S(c+1) ahead of AV(c): the PE queue never waits
                        # on exp(c)
                        et = exp_step(s_step(0))
                        for c in range(NC - 1):
                            sps = s_step(c + 1)
                            av_step(c, et)
                            et = exp_step(sps)
                        av_step(NC - 1, et)
                        # normalize rows 0..63 by row 64 (softmax sum);
                        # copy BOTH psum banks out first so they free before
                        # the recip/mul chain occupies the DVE queue
                        ocps = []
                        for ops in (o_lo, o_hi):
                            ocp = dpool.tile([DH + 1, STRIP], F16, tag="ocp")
                            nc.vector.tensor_copy(ocp[:], ops[:])
                            ocps.append(ocp)
                        for half in (0, 1):
                            ocp = ocps[half]
                            den = dpool.tile([1, STRIP], F16, tag="den")
                            with nc.allow_low_precision(reason="f16 den"):
                                nc.vector.reciprocal(
                                    out=den[:], in_=ocp[DH : DH + 1, :]
                                )
                            rsb2 = dpool.tile([DH, STRIP], F16, tag="rsb2")
                            nc.gpsimd.partition_broadcast(
                                rsb2[:], den[:], channels=DH
                            )
                            nc.vector.tensor_mul(
                                o_t[p][half * DH : (half + 1) * DH, ssl],
                                ocp[0:DH, :],
                                rsb2[:],
                            )

                # pipeline: headnorm(p+1) emitted before attention(p)
                for p in range(NPAIR):
                    if p + 1 < NPAIR:
                        headnorm_rotary(
                            lambda c, p=p: wq_sb[:, c, (p + 1) * 128 : (p + 2) * 128],
                            qr_t[p + 1],
                            gq_sb[:, p + 1 : p + 2],
                        )
                    attention(p)

        # ---------------- phase E: out-projection ----------------
        with ExitStack() as ee:
            wo_pool = ee.enter_context(tc.tile_pool(name="wop", bufs=1))
            outp = ee.enter_context(tc.tile_pool(name="outp", bufs=2))
            xps_pool = ee.enter_context(
                tc.tile_pool(name="xps", bufs=2, space="PSUM")
            )
            wo_sb = wo_pool.tile([128, NPAIR, DIM], F16)
            nc.sync.dma_start(
                out=wo_sb[:], in_=t_wo.rearrange("(p o) d -> o p d", o=128)
            )
            for tb in range(NT):
                xps = xps_pool.tile([128, DIM], F32, tag="xps")
                for p in range(NPAIR):
                    for s in range(4):
                        nc.tensor.matmul(
                            xps[:, s * 512 : (s + 1) * 512],
                            o_t[p][:, tb * 128 : (tb + 1) * 128],
                            wo_sb[:, p, s * 512 : (s + 1) * 512],
                            start=(p == 0),
                            stop=(p == NPAIR - 1),
                        )
                osb = outp.tile([128, DIM], F32, tag="osb")
                nc.vector.tensor_copy(osb[:], xps[:])
                nc.sync.dma_start(
                    out=t_out[tb * 128 : (tb + 1) * 128, :], in_=osb[:]
                )

    nc.compile()
    return nc


def _core_inputs(core, tokens, rotary, norm_w, Wq, Wkv, Wo, gamma_q, gamma_k):
    b, g = core // TPG, core % TPG
    # pair order: pair p = (8g+p, 8g+4+p); lo half -> kv head 2g, hi -> 2g+1
    heads = []
    for p in range(NPAIR):
        heads += [QH_PER * g + p, QH_PER * g + NPAIR + p]
    cols = np.concatenate([np.arange(h * DH, (h + 1) * DH) for h in heads])

    nw = norm_w[:, None].astype(np.float32)
    wq = (Wq * nw)[:, cols].astype(np.float16)
    kcols = np.arange(2 * g * DH, (2 * g + 2) * DH)
    wk = (Wkv[:, : KVH * DH] * nw)[:, kcols].astype(np.float16)
    wv = (Wkv[:, KVH * DH :] * nw)[:, kcols].astype(np.float16)
    wo = Wo[cols, :].astype(np.float16)

    gq = np.empty((NPAIR * 128, 1), np.float32)
    for p in range(NPAIR):
        gq[p * 128 : p * 128 + DH, 0] = (gamma_q[heads[2 * p]] + 1.0) * DH**0.5
        gq[p * 128 + DH : (p + 1) * 128, 0] = (
            gamma_q[heads[2 * p + 1]] + 1.0
        ) * DH**0.5
    gk = np.empty((128, 1), np.float32)
    gk[:DH, 0] = (gamma_k[2 * g] + 1.0) * DH**0.5
    gk[DH:, 0] = (gamma_k[2 * g + 1] + 1.0) * DH**0.5

    cosT = np.cos(rotary).T.astype(np.float32)  # [64, N]
    sinT = np.sin(rotary).T.astype(np.float32)
    sinS = np.concatenate([-sinT[:32], sinT[32:]], axis=0)
    cos_t = np.tile(cosT, (2, 1)).astype(np.float16)
    sins_t = np.tile(sinS, (2, 1)).astype(np.float16)

    sel2 = np.zeros((128, 2), np.float16)
    sel2[:DH, 0] = 1
    sel2[DH:, 1] = 1
    bc2 = np.zeros((2, 128), np.float32)
    bc2[0, :DH] = 1
    bc2[1, DH:] = 1

    return {
        "tokens": np.ascontiguousarray(tokens[b]),
        "wq": np.ascontiguousarray(wq),
        "wk": np.ascontiguousarray(wk),
        "wv": np.ascontiguousarray(wv),
        "wo": np.ascontiguousarray(wo),
        "gq": gq,
        "gk": gk,
        "cos_t": cos_t,
        "sins_t": sins_t,
        "sel2": sel2,
        "bc2": bc2,
        "ones_col": np.ones((1, DH), np.float32),
        "ident": np.eye(128, dtype=np.float16),
    }


def kernel(tokens, rotary, norm_w, Wq, Wkv, Wo, gamma_q, gamma_k, _bench=None):
    if "nc" not in _BUILD:
        _BUILD["nc"] = _build_nc()
    nc = _BUILD["nc"]

    in_maps = [
        _core_inputs(c, tokens, rotary, norm_w, Wq, Wkv, Wo, gamma_q, gamma_k)
        for c in range(NCORES)
    ]
    kw = dict(_bench or {})
    res = run_bass_kernel_spmd(nc, in_maps, list(range(NCORES)), **kw)
    if _bench is not None:
        _BUILD["last"] = res

    out = np.empty((B, N, DIM), np.float32)
    for b in range(B):
        acc = res.results[b * TPG]["out"].astype(np.float32)
        for g in range(1, TPG):
            acc = acc + res.results[b * TPG + g]["out"]
        out[b] = acc
    return out



# revision 4
# speedup vs baseline: 1.0050x; 1.0029x over previous
"""GQA attention block (rmsnorm + qk-headnorm + rotary + softmax + out-proj)
for Trainium2, SPMD over 8 NeuronCores: 2-way data parallel (batch) x 4-way
tensor parallel (query-head groups). Partial out-proj results are summed on
host (the TP all-reduce).

Shapes (hardcoded): tokens [2,2048,2048] f32, rotary [2048,64], norm_w [2048],
Wq [2048,2048], Wkv [2048,1024], Wo [2048,2048], gamma_q [32,64], gamma_k [8,64].

Per-core layout: pairs of heads packed on 128 partitions (2 x 64d). Attention
runs transposed (S^T = K^T-chunk @ Q^T strip), V is ones-augmented so the
softmax denominator falls out of the AV matmul's 65th row. Emission interleaves
pair p's attention (ACT-bound exp) with pair p+1's projections (PE-bound).
"""

import sys

for _p in ("/opt/trn_rl_repo", "/opt/pypackages"):
    if _p not in sys.path:
        sys.path.append(_p)

from contextlib import ExitStack

import numpy as np

import concourse.bass as bass
import concourse.tile as tile
from concourse import bacc, mybir
from concourse.bass_utils import run_bass_kernel_spmd
from concourse import hw_specs as _hw_specs
from concourse import bacc as _bacc_mod

_orig_get_tables = _hw_specs.get_activation_tables.__wrapped__


def _tables_nle_first(arch):
    # Keep set order/ids (walrus maps ids by position); empty the others so
    # the first-match chooser always lands on the set holding exp+ln+square,
    # eliminating mid-kernel ACT table reloads.
    t = _orig_get_tables(arch)
    keep = "natural_log_exp_and_others"
    return {k: (v if k == keep else set()) for k, v in t.items()}


_hw_specs.get_activation_tables = _tables_nle_first
_bacc_mod.get_activation_tables = _tables_nle_first

B, N, DIM = 2, 2048, 2048
DH = 64  # head dim
QH, KVH = 32, 8
NCORES = 8
TPG = 4  # tensor-parallel groups
QH_PER = QH // TPG  # 8 q heads per core
NPAIR = QH_PER // 2  # 4 pairs of q heads packed 2-per-128-partitions
NT = N // 128  # 16 token tiles
NC = DIM // 128  # 16 contraction chunks
STRIP = 512  # q-strip width in attention
NSTRIP = N // STRIP

F32 = mybir.dt.float32
F32R = mybir.dt.float32r
F16 = mybir.dt.float16

_BUILD = {}


def _build_nc():
    """Trace + compile the per-core Bass kernel (same program all cores)."""
    nc = bacc.Bacc(
        "TRN2", target_bir_lowering=False, debug=False, num_devices=NCORES
    )

    t_tokens = nc.dram_tensor("tokens", [N, DIM], F32, kind="ExternalInput").ap()
    t_wq = nc.dram_tensor("wq", [DIM, QH_PER * DH], F16, kind="ExternalInput").ap()
    t_wk = nc.dram_tensor("wk", [DIM, 2 * DH], F16, kind="ExternalInput").ap()
    t_wv = nc.dram_tensor("wv", [DIM, 2 * DH], F16, kind="ExternalInput").ap()
    t_wo = nc.dram_tensor("wo", [QH_PER * DH, DIM], F16, kind="ExternalInput").ap()
    t_gq = nc.dram_tensor("gq", [NPAIR * 128, 1], F32, kind="ExternalInput").ap()
    t_gk = nc.dram_tensor("gk", [128, 1], F32, kind="ExternalInput").ap()
    t_cos = nc.dram_tensor("cos_t", [128, N], F16, kind="ExternalInput").ap()
    t_sin = nc.dram_tensor("sins_t", [128, N], F16, kind="ExternalInput").ap()
    t_sel2 = nc.dram_tensor("sel2", [128, 2], F16, kind="ExternalInput").ap()
    t_bc2 = nc.dram_tensor("bc2", [2, 128], F32R, kind="ExternalInput").ap()
    t_ones = nc.dram_tensor("ones_col", [1, DH], F32R, kind="ExternalInput").ap()
    t_ident = nc.dram_tensor("ident", [128, 128], F16, kind="ExternalInput").ap()
    t_out = nc.dram_tensor("out", [N, DIM], F32, kind="ExternalOutput").ap()

    with tile.TileContext(nc) as tc, ExitStack() as ctx:
        # ---------------- persistent pools (whole kernel) ----------------
        persist = ctx.enter_context(tc.tile_pool(name="persist", bufs=1))
        qr_pool = ctx.enter_context(tc.tile_pool(name="qrp", bufs=1))

        # small constants
        eps_t = persist.tile([128, 1], F32)
        nc.vector.memset(eps_t, float(np.finfo(np.float32).eps))
        tiny_t = persist.tile([128, 1], F32)
        nc.vector.memset(tiny_t, 1e-24)
        zero_t = persist.tile([128, 1], F32)
        nc.vector.memset(zero_t, 0.0)
        sel2_sb = persist.tile([128, 2], F16)
        nc.sync.dma_start(out=sel2_sb[:], in_=t_sel2)
        bc2_sb = persist.tile([2, 128], F32R)
        nc.sync.dma_start(out=bc2_sb[:], in_=t_bc2)
        ones_sb = persist.tile([1, DH], F32R)
        nc.sync.dma_start(out=ones_sb[:], in_=t_ones)
        ident_sb = persist.tile([128, 128], F16)
        nc.sync.dma_start(out=ident_sb[:], in_=t_ident)
        gq_sb = persist.tile([128, NPAIR], F32)
        nc.sync.dma_start(
            out=gq_sb[:], in_=t_gq.rearrange("(p o) u -> o (p u)", o=128)
        )
        gk_sb = persist.tile([128, 1], F32)
        nc.sync.dma_start(out=gk_sb[:], in_=t_gk)

        # rotated q/k in f16, [2 heads * 64d, ntok]
        qr_t = [
            qr_pool.tile([128, N], F16, tag=f"qr{p}", name=f"qr{p}")
            for p in range(NPAIR)
        ]
        kr_t = qr_pool.tile([128, N], F16, tag="kr")
        # augmented V (natural layout): [tok-part, chunk, [v0|1|v1|1]]
        v_all = qr_pool.tile([128, NT, 2 * DH + 2], F16, tag="vall")
        nc.vector.memset(v_all[:, :, DH : DH + 1], 1.0)
        nc.vector.memset(v_all[:, :, 2 * DH + 1 : 2 * DH + 2], 1.0)
        # normalized attention outputs per pair [2 heads * 64d, ntok]
        o_t = [
            qr_pool.tile([128, N], F16, tag=f"o{p}", name=f"o{p}")
            for p in range(NPAIR)
        ]

        with ExitStack() as abc:
            xt_pool = abc.enter_context(tc.tile_pool(name="xtp", bufs=1))
            # x^T, f16, per 512-token strip: [dim-in-chunk-part, chunk, 512]
            xt_s = [
                xt_pool.tile([128, NC, 512], F16, tag=f"xt{s}", name=f"xt{s}")
                for s in range(4)
            ]

            with ExitStack() as bb:
                apool = bb.enter_context(tc.tile_pool(name="apool", bufs=2))
                spool = bb.enter_context(tc.tile_pool(name="spool", bufs=3))
                wpool = bb.enter_context(tc.tile_pool(name="wpool", bufs=1))
                cpool = bb.enter_context(tc.tile_pool(name="cpool", bufs=1))
                hpool = bb.enter_context(tc.tile_pool(name="hpool", bufs=2))
                epool = bb.enter_context(tc.tile_pool(name="epool", bufs=3))
                dpool = bb.enter_context(tc.tile_pool(name="dpool", bufs=2))
                # PSUM budget (8 banks): sps 2x2 (S-tiles + q-proj acc)
                # + ops 2 (o-acc, v-acc, transposes) + bc 2 (hsum/bcast/rps
                # + transposes)
                bcast_pool = bb.enter_context(
                    tc.tile_pool(name="bcp", bufs=2, space="PSUM")
                )
                drpool = bb.enter_context(
                    tc.tile_pool(name="drp", bufs=4, space="DRAM")
                )
                sps_pool = bb.enter_context(
                    tc.tile_pool(name="sps", bufs=2, space="PSUM")
                )
                ops_pool = bb.enter_context(
                    tc.tile_pool(name="ops", bufs=1, space="PSUM")
                )

                cos_sb = wpool.tile([128, N], F16)
                nc.gpsimd.dma_start(out=cos_sb[:], in_=t_cos)
                sin_sb = wpool.tile([128, N], F16)
                nc.gpsimd.dma_start(out=sin_sb[:], in_=t_sin)
                wq_sb = wpool.tile([128, NC, QH_PER * DH], F16)
                nc.gpsimd.dma_start(
                    out=wq_sb[:], in_=t_wq.rearrange("(c p) q -> p c q", p=128)
                )
                wk_sb = wpool.tile([128, NC, 2 * DH], F16)
                nc.gpsimd.dma_start(
                    out=wk_sb[:], in_=t_wk.rearrange("(c p) q -> p c q", p=128)
                )
                wv_sb = wpool.tile([128, NC, 2 * DH], F16)
                nc.gpsimd.dma_start(
                    out=wv_sb[:], in_=t_wv.rearrange("(c p) q -> p c q", p=128)
                )

                def a_tile(tt):
                    """rmsnorm + transpose token-tile tt into xt_s."""
                    x_nat = apool.tile([128, DIM], F32, tag="xnat", bufs=3)
                    nc.sync.dma_start(
                        out=x_nat[:], in_=t_tokens[tt * 128 : (tt + 1) * 128, :]
                    )
                    xs = apool.tile([128, DIM], F16, tag="xs")
                    ssum = spool.tile([128, 1], F32, tag="ssum")
                    nc.scalar.activation(
                        out=xs[:],
                        in_=x_nat[:],
                        func=mybir.ActivationFunctionType.Square,
                        bias=zero_t[:],
                        accum_out=ssum[:],
                    )
                    lns = spool.tile([128, 1], F32, tag="lns")
                    nc.scalar.activation(
                        out=lns[:],
                        in_=ssum[:],
                        func=mybir.ActivationFunctionType.Ln,
                        bias=eps_t[:],
                        scale=1.0 / DIM,
                    )
                    rstd = spool.tile([128, 1], F32, tag="rstd")
                    nc.scalar.activation(
                        out=rstd[:],
                        in_=lns[:],
                        func=mybir.ActivationFunctionType.Exp,
                        bias=zero_t[:],
                        scale=-0.5,
                    )
                    nc.vector.tensor_scalar_mul(xs[:], x_nat[:], rstd[:])
                    st, off = tt // 4, (tt % 4) * 128
                    for cg in range(4):
                        if cg % 2 == 0:
                            tp = ops_pool.tile(
                                [128, 4, 128], F16, tag="ohi", name="tp"
                            )
                        else:
                            tp = bcast_pool.tile(
                                [128, 4, 128], F16, tag="bc", name="tp"
                            )
                        for j in range(4):
                            c = 4 * cg + j
                            nc.tensor.transpose(
                                tp[:, j, :],
                                xs[:, c * 128 : (c + 1) * 128],
                                ident_sb[:],
                            )
                        nc.vector.tensor_copy(
                            xt_s[st][:, 4 * cg : 4 * cg + 4, off : off + 128],
                            tp[:],
                        )

                def proj_norm_slice(proj_w, qn, s, gscale_ap):
                    """Project slice s, head-l2norm + gamma into qn[:, s]."""
                    ssl = slice(s * 512, (s + 1) * 512)
                    qps = sps_pool.tile([128, 512], F32, tag="sps", name="qps")
                    for c in range(NC):
                        nc.tensor.matmul(
                            qps[:],
                            proj_w(c),
                            xt_s[s][:, c, :],
                            start=(c == 0),
                            stop=(c == NC - 1),
                        )
                    sq2 = hpool.tile([128, 512], F16, tag="sq2")
                    nc.scalar.activation(
                        out=sq2[:],
                        in_=qps[:],
                        func=mybir.ActivationFunctionType.Square,
                        bias=zero_t[:],
                    )
                    hsum = bcast_pool.tile([2, 512], F32, tag="bc", name="hsum")
                    nc.tensor.matmul(
                        hsum[:], sel2_sb[:], sq2[:], start=True, stop=True
                    )
                    lnh = hpool.tile([2, 512], F32, tag="lnh", bufs=1)
                    nc.scalar.activation(
                        out=lnh[:],
                        in_=hsum[:],
                        func=mybir.ActivationFunctionType.Ln,
                        bias=tiny_t[0:2],
                    )
                    hrec = hpool.tile([2, 512], F32R, tag="hrec")
                    nc.scalar.activation(
                        out=hrec[:],
                        in_=lnh[:],
                        func=mybir.ActivationFunctionType.Exp,
                        bias=zero_t[0:2],
                        scale=-0.5,
                    )
                    bps = bcast_pool.tile([128, 512], F32, tag="bc")
                    nc.tensor.matmul(
                        bps[:], bc2_sb[:], hrec[:], start=True, stop=True
                    )
                    rsb = hpool.tile([128, 512], F16, tag="rsb")
                    nc.vector.tensor_scalar_mul(rsb[:], bps[:], gscale_ap)
                    nc.vector.tensor_mul(qn[:, ssl], qps[:], rsb[:])

                def rotary_finish(qn, dst):
                    # rotate-half swap within each head (partition +-32)
                    qsw = cpool.tile([128, N], F16, tag="qsw", bufs=1)
                    for h0 in (0, 64):
                        nc.sync.dma_start(
                            out=qsw[h0 : h0 + 32, :], in_=qn[h0 + 32 : h0 + 64, :]
                        )
                        nc.sync.dma_start(
                            out=qsw[h0 + 32 : h0 + 64, :], in_=qn[h0 : h0 + 32, :]
                        )
                    nc.vector.tensor_mul(qn[:], qn[:], cos_sb[:])
                    nc.vector.tensor_mul(qsw[:], qsw[:], sin_sb[:])
                    nc.vector.tensor_add(dst[:], qn[:], qsw[:])

                def headnorm_rotary(proj_w, dst, gscale_ap):
                    qn = cpool.tile([128, N], F16, tag="qn", bufs=2, name="qn")
                    for s in range(4):
                        proj_norm_slice(proj_w, qn, s, gscale_ap)
                    rotary_finish(qn, dst)

                def v_tile(tb):
                    vps = ops_pool.tile([128, 2 * DH], F32, tag="olo", name="vps")
                    for c in range(NC):
                        nc.tensor.matmul(
                            vps[:],
                            xt_s[tb // 4][:, c, (tb % 4) * 128 : (tb % 4 + 1) * 128],
                            wv_sb[:, c, :],
                            start=(c == 0),
                            stop=(c == NC - 1),
                        )
                    nc.vector.tensor_copy(v_all[:, tb, 0:DH], vps[:, 0:DH])
                    nc.vector.tensor_copy(
                        v_all[:, tb, DH + 1 : 2 * DH + 1], vps[:, DH : 2 * DH]
                    )

                # ---- interleaved prologue: per strip, A-tiles then K/V/Q0
                qn_k = cpool.tile([128, N], F16, tag="qn", bufs=2, name="qn_k")
                qn_q0 = cpool.tile([128, N], F16, tag="qn", bufs=2, name="qn_q0")
                for s in range(4):
                    for tt in range(4 * s, 4 * s + 4):
                        a_tile(tt)
                    proj_norm_slice(lambda c: wk_sb[:, c, :], qn_k, s, gk_sb[:])
                    for tb in range(4 * s, 4 * s + 4):
                        v_tile(tb)
                    proj_norm_slice(
                        lambda c: wq_sb[:, c, 0:128], qn_q0, s, gq_sb[:, 0:1]
                    )
                rotary_finish(qn_k, kr_t)
                rotary_finish(qn_q0, qr_t[0])

                def attention(p):
                    for st in range(NSTRIP):
                        ssl = slice(st * STRIP, (st + 1) * STRIP)
                        o_lo = ops_pool.tile([DH + 1, STRIP], F32, tag="olo")
                        o_hi = ops_pool.tile([DH + 1, STRIP], F32, tag="ohi")

                        def s_step(c):
                            ck = slice(c * 128, (c + 1) * 128)
                            sps = sps_pool.tile(
                                [128, 2, STRIP], F32, tag="sps"
                            )
                            nc.tensor.matmul(
                                sps[:, 0, :],
                                kr_t[0:DH, ck],
                                qr_t[p][0:DH, ssl],
                                start=True,
                                stop=True,
                            )
                            nc.tensor.matmul(
                                sps[:, 1, :],
                                kr_t[DH:128, ck],
                                qr_t[p][DH:128, ssl],
                                start=True,
                                stop=True,
                            )
                            return sps

                        def exp_step(sps):
                            et = epool.tile([128, 2, STRIP], F16, tag="et")
                            nc.scalar.activation(
                                out=et[:],
                                in_=sps[:],
                                func=mybir.ActivationFunctionType.Exp,
                                bias=zero_t[:],
                                scale=float(DH) ** -0.5,
                            )
                            return et

                        def av_step(c, et):
                            nc.tensor.matmul(
                                o_lo[:],
                                v_all[:, c, 0 : DH + 1],
                                et[:, 0, :],
                                start=(c == 0),
                                stop=(c == NC - 1),
                            )
                            nc.tensor.matmul(
                                o_hi[:],
                                v_all[:, c, DH + 1 : 2 * DH + 2],
                                et[:, 1, :],
                                start=(c == 0),
                                stop=(c == NC - 1),
                            )

                        # S(c+1) ahead of AV(c): the PE queue never waits
                        # on exp(c)
                        et = exp_step(s_step(0))
                        for c in range(NC - 1):
                            sps = s_step(c + 1)
                            av_step(c, et)
                            et = exp_step(sps)
                        av_step(NC - 1, et)
                        # normalize rows 0..63 by row 64 (softmax sum);
                        # copy BOTH psum banks out first so they free before
                        # the recip/mul chain occupies the DVE queue
                        ocps = []
                        for ops in (o_lo, o_hi):
                            ocp = dpool.tile([DH + 1, STRIP], F16, tag="ocp")
                            nc.vector.tensor_copy(ocp[:], ops[:])
                            ocps.append(ocp)
                        for half in (0, 1):
                            ocp = ocps[half]
                            den = dpool.tile([1, STRIP], F16, tag="den")
                            with nc.allow_low_precision(reason="f16 den"):
                                nc.vector.reciprocal(
                                    out=den[:], in_=ocp[DH : DH + 1, :]
                                )
                            rsb2 = dpool.tile([DH, STRIP], F16, tag="rsb2")
                            nc.gpsimd.partition_broadcast(
                                rsb2[:], den[:], channels=DH
                            )
                            nc.vector.tensor_mul(
                                o_t[p][half * DH : (half + 1) * DH, ssl],
                                ocp[0:DH, :],
                                rsb2[:],
                            )

                # pipeline: headnorm(p+1) emitted before attention(p)
                for p in range(NPAIR):
                    if p + 1 < NPAIR:
                        headnorm_rotary(
                            lambda c, p=p: wq_sb[:, c, (p + 1) * 128 : (p + 2) * 128],
                            qr_t[p + 1],
                            gq_sb[:, p + 1 : p + 2],
                        )
                    attention(p)

        # ---------------- phase E: out-projection ----------------
        with ExitStack() as ee:
            wo_pool = ee.enter_context(tc.tile_pool(name="wop", bufs=1))
            outp = ee.enter_context(tc.tile_pool(name="outp", bufs=2))
            xps_pool = ee.enter_context(
                tc.tile_pool(name="xps", bufs=2, space="PSUM")
            )
            wo_sb = wo_pool.tile([128, NPAIR, DIM], F16)
            nc.sync.dma_start(
                out=wo_sb[:], in_=t_wo.rearrange("(p o) d -> o p d", o=128)
            )
            for tb in range(NT):
                xps = xps_pool.tile([128, DIM], F32, tag="xps")
                for p in range(NPAIR):
                    for s in range(4):
                        nc.tensor.matmul(
                            xps[:, s * 512 : (s + 1) * 512],
                            o_t[p][:, tb * 128 : (tb + 1) * 128],
                            wo_sb[:, p, s * 512 : (s + 1) * 512],
                            start=(p == 0),
                            stop=(p == NPAIR - 1),
                        )
                osb = outp.tile([128, DIM], F32, tag="osb")
                nc.vector.tensor_copy(osb[:], xps[:])
                nc.sync.dma_start(
                    out=t_out[tb * 128 : (tb + 1) * 128, :], in_=osb[:]
                )

    nc.compile()
    return nc


def _core_inputs(core, tokens, rotary, norm_w, Wq, Wkv, Wo, gamma_q, gamma_k):
    b, g = core // TPG, core % TPG
    # pair order: pair p = (8g+p, 8g+4+p); lo half -> kv head 2g, hi -> 2g+1
    heads = []
    for p in range(NPAIR):
        heads += [QH_PER * g + p, QH_PER * g + NPAIR + p]
    cols = np.concatenate([np.arange(h * DH, (h + 1) * DH) for h in heads])

    nw = norm_w[:, None].astype(np.float32)
    wq = (Wq * nw)[:, cols].astype(np.float16)
    kcols = np.arange(2 * g * DH, (2 * g + 2) * DH)
    wk = (Wkv[:, : KVH * DH] * nw)[:, kcols].astype(np.float16)
    wv = (Wkv[:, KVH * DH :] * nw)[:, kcols].astype(np.float16)
    wo = Wo[cols, :].astype(np.float16)

    gq = np.empty((NPAIR * 128, 1), np.float32)
    for p in range(NPAIR):
        gq[p * 128 : p * 128 + DH, 0] = (gamma_q[heads[2 * p]] + 1.0) * DH**0.5
        gq[p * 128 + DH : (p + 1) * 128, 0] = (
            gamma_q[heads[2 * p + 1]] + 1.0
        ) * DH**0.5
    gk = np.empty((128, 1), np.float32)
    gk[:DH, 0] = (gamma_k[2 * g] + 1.0) * DH**0.5
    gk[DH:, 0] = (gamma_k[2 * g + 1] + 1.0) * DH**0.5

    cosT = np.cos(rotary).T.astype(np.float32)  # [64, N]
    sinT = np.sin(rotary).T.astype(np.float32)
    sinS = np.concatenate([-sinT[:32], sinT[32:]], axis=0)
    cos_t = np.tile(cosT, (2, 1)).astype(np.float16)
    sins_t = np.tile(sinS, (2, 1)).astype(np.float16)

    sel2 = np.zeros((128, 2), np.float16)
    sel2[:DH, 0] = 1
    sel2[DH:, 1] = 1
    bc2 = np.zeros((2, 128), np.float32)
    bc2[0, :DH] = 1
    bc2[1, DH:] = 1

    return {
        "tokens": np.ascontiguousarray(tokens[b]),
        "wq": np.ascontiguousarray(wq),
        "wk": np.ascontiguousarray(wk),
        "wv": np.ascontiguousarray(wv),
        "wo": np.ascontiguousarray(wo),
        "gq": gq,
        "gk": gk,
        "cos_t": cos_t,
        "sins_t": sins_t,
        "sel2": sel2,
        "bc2": bc2,
        "ones_col": np.ones((1, DH), np.float32),
        "ident": np.eye(128, dtype=np.float16),
    }


def kernel(tokens, rotary, norm_w, Wq, Wkv, Wo, gamma_q, gamma_k, _bench=None):
    if "nc" not in _BUILD:
        _BUILD["nc"] = _build_nc()
    nc = _BUILD["nc"]

    in_maps = [
        _core_inputs(c, tokens, rotary, norm_w, Wq, Wkv, Wo, gamma_q, gamma_k)
        for c in range(NCORES)
    ]
    kw = dict(_bench or {})
    res = run_bass_kernel_spmd(nc, in_maps, list(range(NCORES)), **kw)
    if _bench is not None:
        _BUILD["last"] = res

    out = np.empty((B, N, DIM), np.float32)
    for b in range(B):
        acc = res.results[b * TPG]["out"].astype(np.float32)
        for g in range(1, TPG):
            acc = acc + res.results[b * TPG + g]["out"]
        out[b] = acc
    return out



# revision 5
# speedup vs baseline: 1.0090x; 1.0040x over previous
"""GQA attention block (rmsnorm + qk-headnorm + rotary + softmax + out-proj)
for Trainium2, SPMD over 8 NeuronCores: 2-way data parallel (batch) x 4-way
tensor parallel (query-head groups). Partial out-proj results are summed on
host (the TP all-reduce).

Shapes (hardcoded): tokens [2,2048,2048] f32, rotary [2048,64], norm_w [2048],
Wq [2048,2048], Wkv [2048,1024], Wo [2048,2048], gamma_q [32,64], gamma_k [8,64].

Per-core layout: pairs of heads packed on 128 partitions (2 x 64d). Attention
runs transposed (S^T = K^T-chunk @ Q^T strip), V is ones-augmented so the
softmax denominator falls out of the AV matmul's 65th row. Emission interleaves
pair p's attention (ACT-bound exp) with pair p+1's projections (PE-bound).
"""

import sys

for _p in ("/opt/trn_rl_repo", "/opt/pypackages"):
    if _p not in sys.path:
        sys.path.append(_p)

from contextlib import ExitStack

import numpy as np

import concourse.bass as bass
import concourse.tile as tile
from concourse import bacc, mybir
from concourse.bass_utils import run_bass_kernel_spmd
from concourse import hw_specs as _hw_specs
from concourse import bacc as _bacc_mod

_orig_get_tables = _hw_specs.get_activation_tables.__wrapped__


def _tables_nle_first(arch):
    # Keep set order/ids (walrus maps ids by position); empty the others so
    # the first-match chooser always lands on the set holding exp+ln+square,
    # eliminating mid-kernel ACT table reloads.
    t = _orig_get_tables(arch)
    keep = "natural_log_exp_and_others"
    return {k: (v if k == keep else set()) for k, v in t.items()}


_hw_specs.get_activation_tables = _tables_nle_first
_bacc_mod.get_activation_tables = _tables_nle_first

B, N, DIM = 2, 2048, 2048
DH = 64  # head dim
QH, KVH = 32, 8
NCORES = 8
TPG = 4  # tensor-parallel groups
QH_PER = QH // TPG  # 8 q heads per core
NPAIR = QH_PER // 2  # 4 pairs of q heads packed 2-per-128-partitions
NT = N // 128  # 16 token tiles
NC = DIM // 128  # 16 contraction chunks
STRIP = 512  # q-strip width in attention
NSTRIP = N // STRIP

F32 = mybir.dt.float32
F32R = mybir.dt.float32r
F16 = mybir.dt.float16

_BUILD = {}


def _build_nc():
    """Trace + compile the per-core Bass kernel (same program all cores)."""
    nc = bacc.Bacc(
        "TRN2", target_bir_lowering=False, debug=False, num_devices=NCORES
    )

    t_tokens = nc.dram_tensor("tokens", [N, DIM], F32, kind="ExternalInput").ap()
    t_wq = nc.dram_tensor("wq", [DIM, QH_PER * DH], F16, kind="ExternalInput").ap()
    t_wk = nc.dram_tensor("wk", [DIM, 2 * DH], F16, kind="ExternalInput").ap()
    t_wv = nc.dram_tensor("wv", [DIM, 2 * DH], F16, kind="ExternalInput").ap()
    t_wo = nc.dram_tensor("wo", [QH_PER * DH, DIM], F16, kind="ExternalInput").ap()
    t_gq = nc.dram_tensor("gq", [NPAIR * 128, 1], F32, kind="ExternalInput").ap()
    t_gk = nc.dram_tensor("gk", [128, 1], F32, kind="ExternalInput").ap()
    t_cos = nc.dram_tensor("cos_t", [128, N], F16, kind="ExternalInput").ap()
    t_sin = nc.dram_tensor("sins_t", [128, N], F16, kind="ExternalInput").ap()
    t_sel2 = nc.dram_tensor("sel2", [128, 2], F16, kind="ExternalInput").ap()
    t_bc2 = nc.dram_tensor("bc2", [2, 128], F32R, kind="ExternalInput").ap()
    t_ones = nc.dram_tensor("ones_col", [1, DH], F32R, kind="ExternalInput").ap()
    t_ident = nc.dram_tensor("ident", [128, 128], F16, kind="ExternalInput").ap()
    t_out = nc.dram_tensor("out", [N, DIM], F32, kind="ExternalOutput").ap()

    with tile.TileContext(nc) as tc, ExitStack() as ctx:
        # ---------------- persistent pools (whole kernel) ----------------
        persist = ctx.enter_context(tc.tile_pool(name="persist", bufs=1))
        qr_pool = ctx.enter_context(tc.tile_pool(name="qrp", bufs=1))

        # small constants
        eps_t = persist.tile([128, 1], F32)
        nc.vector.memset(eps_t, float(np.finfo(np.float32).eps))
        tiny_t = persist.tile([128, 1], F32)
        nc.vector.memset(tiny_t, 1e-24)
        zero_t = persist.tile([128, 1], F32)
        nc.vector.memset(zero_t, 0.0)
        sel2_sb = persist.tile([128, 2], F16)
        nc.sync.dma_start(out=sel2_sb[:], in_=t_sel2)
        bc2_sb = persist.tile([2, 128], F32R)
        nc.sync.dma_start(out=bc2_sb[:], in_=t_bc2)
        ones_sb = persist.tile([1, DH], F32R)
        nc.sync.dma_start(out=ones_sb[:], in_=t_ones)
        ident_sb = persist.tile([128, 128], F16)
        nc.sync.dma_start(out=ident_sb[:], in_=t_ident)
        gq_sb = persist.tile([128, NPAIR], F32)
        nc.sync.dma_start(
            out=gq_sb[:], in_=t_gq.rearrange("(p o) u -> o (p u)", o=128)
        )
        gk_sb = persist.tile([128, 1], F32)
        nc.sync.dma_start(out=gk_sb[:], in_=t_gk)

        # rotated q/k in f16, [2 heads * 64d, ntok]
        qr_t = [
            qr_pool.tile([128, N], F16, tag=f"qr{p}", name=f"qr{p}")
            for p in range(NPAIR)
        ]
        kr_t = qr_pool.tile([128, N], F16, tag="kr")
        # augmented V (natural layout): [tok-part, chunk, [v0|1|v1|1]]
        v_all = qr_pool.tile([128, NT, 2 * DH + 2], F16, tag="vall")
        nc.vector.memset(v_all[:, :, DH : DH + 1], 1.0)
        nc.vector.memset(v_all[:, :, 2 * DH + 1 : 2 * DH + 2], 1.0)
        # normalized attention outputs per pair [2 heads * 64d, ntok]
        o_t = [
            qr_pool.tile([128, N], F16, tag=f"o{p}", name=f"o{p}")
            for p in range(NPAIR)
        ]

        with ExitStack() as abc:
            xt_pool = abc.enter_context(tc.tile_pool(name="xtp", bufs=1))
            # x^T, f16, per 512-token strip: [dim-in-chunk-part, chunk, 512]
            xt_s = [
                xt_pool.tile([128, NC, 512], F16, tag=f"xt{s}", name=f"xt{s}")
                for s in range(4)
            ]

            with ExitStack() as bb:
                apool = bb.enter_context(tc.tile_pool(name="apool", bufs=2))
                spool = bb.enter_context(tc.tile_pool(name="spool", bufs=3))
                wpool = bb.enter_context(tc.tile_pool(name="wpool", bufs=1))
                cpool = bb.enter_context(tc.tile_pool(name="cpool", bufs=1))
                hpool = bb.enter_context(tc.tile_pool(name="hpool", bufs=2))
                epool = bb.enter_context(tc.tile_pool(name="epool", bufs=3))
                dpool = bb.enter_context(tc.tile_pool(name="dpool", bufs=2))
                # PSUM budget (8 banks): sps 2x2 (S-tiles + q-proj acc)
                # + ops 2 (o-acc, v-acc, transposes) + bc 2 (hsum/bcast/rps
                # + transposes)
                bcast_pool = bb.enter_context(
                    tc.tile_pool(name="bcp", bufs=2, space="PSUM")
                )
                drpool = bb.enter_context(
                    tc.tile_pool(name="drp", bufs=4, space="DRAM")
                )
                sps_pool = bb.enter_context(
                    tc.tile_pool(name="sps", bufs=2, space="PSUM")
                )
                ops_pool = bb.enter_context(
                    tc.tile_pool(name="ops", bufs=1, space="PSUM")
                )

                cos_sb = wpool.tile([128, N], F16)
                nc.gpsimd.dma_start(out=cos_sb[:], in_=t_cos)
                sin_sb = wpool.tile([128, N], F16)
                nc.gpsimd.dma_start(out=sin_sb[:], in_=t_sin)
                wq_sb = wpool.tile([128, NC, QH_PER * DH], F16)
                nc.gpsimd.dma_start(
                    out=wq_sb[:], in_=t_wq.rearrange("(c p) q -> p c q", p=128)
                )
                wk_sb = wpool.tile([128, NC, 2 * DH], F16)
                nc.gpsimd.dma_start(
                    out=wk_sb[:], in_=t_wk.rearrange("(c p) q -> p c q", p=128)
                )
                wv_sb = wpool.tile([128, NC, 2 * DH], F16)
                nc.gpsimd.dma_start(
                    out=wv_sb[:], in_=t_wv.rearrange("(c p) q -> p c q", p=128)
                )

                def a_tile(tt):
                    """rmsnorm + transpose token-tile tt into xt_s."""
                    x_nat = apool.tile([128, DIM], F32, tag="xnat", bufs=3)
                    nc.sync.dma_start(
                        out=x_nat[:], in_=t_tokens[tt * 128 : (tt + 1) * 128, :]
                    )
                    xs = apool.tile([128, DIM], F16, tag="xs")
                    ssum = spool.tile([128, 1], F32, tag="ssum")
                    nc.scalar.activation(
                        out=xs[:],
                        in_=x_nat[:],
                        func=mybir.ActivationFunctionType.Square,
                        bias=zero_t[:],
                        accum_out=ssum[:],
                    )
                    lns = spool.tile([128, 1], F32, tag="lns")
                    nc.scalar.activation(
                        out=lns[:],
                        in_=ssum[:],
                        func=mybir.ActivationFunctionType.Ln,
                        bias=eps_t[:],
                        scale=1.0 / DIM,
                    )
                    rstd = spool.tile([128, 1], F32, tag="rstd")
                    nc.scalar.activation(
                        out=rstd[:],
                        in_=lns[:],
                        func=mybir.ActivationFunctionType.Exp,
                        bias=zero_t[:],
                        scale=-0.5,
                    )
                    nc.vector.tensor_scalar_mul(
                        xs[:, 0:1024], x_nat[:, 0:1024], rstd[:]
                    )
                    nc.gpsimd.tensor_scalar_mul(
                        xs[:, 1024:2048], x_nat[:, 1024:2048], rstd[:]
                    )
                    st, off = tt // 4, (tt % 4) * 128
                    for cg in range(4):
                        if cg % 2 == 0:
                            tp = ops_pool.tile(
                                [128, 4, 128], F16, tag="ohi", name="tp"
                            )
                        else:
                            tp = bcast_pool.tile(
                                [128, 4, 128], F16, tag="bc", name="tp"
                            )
                        for j in range(4):
                            c = 4 * cg + j
                            nc.tensor.transpose(
                                tp[:, j, :],
                                xs[:, c * 128 : (c + 1) * 128],
                                ident_sb[:],
                            )
                        nc.vector.tensor_copy(
                            xt_s[st][:, 4 * cg : 4 * cg + 4, off : off + 128],
                            tp[:],
                        )

                def proj_norm_slice(proj_w, qn, s, gscale_ap):
                    """Project slice s, head-l2norm + gamma into qn[:, s]."""
                    ssl = slice(s * 512, (s + 1) * 512)
                    qps = sps_pool.tile([128, 512], F32, tag="sps", name="qps")
                    for c in range(NC):
                        nc.tensor.matmul(
                            qps[:],
                            proj_w(c),
                            xt_s[s][:, c, :],
                            start=(c == 0),
                            stop=(c == NC - 1),
                        )
                    sq2 = hpool.tile([128, 512], F16, tag="sq2")
                    nc.scalar.activation(
                        out=sq2[:],
                        in_=qps[:],
                        func=mybir.ActivationFunctionType.Square,
                        bias=zero_t[:],
                    )
                    hsum = bcast_pool.tile([2, 512], F32, tag="bc", name="hsum")
                    nc.tensor.matmul(
                        hsum[:], sel2_sb[:], sq2[:], start=True, stop=True
                    )
                    lnh = hpool.tile([2, 512], F32, tag="lnh", bufs=1)
                    nc.scalar.activation(
                        out=lnh[:],
                        in_=hsum[:],
                        func=mybir.ActivationFunctionType.Ln,
                        bias=tiny_t[0:2],
                    )
                    hrec = hpool.tile([2, 512], F32R, tag="hrec")
                    nc.scalar.activation(
                        out=hrec[:],
                        in_=lnh[:],
                        func=mybir.ActivationFunctionType.Exp,
                        bias=zero_t[0:2],
                        scale=-0.5,
                    )
                    bps = bcast_pool.tile([128, 512], F32, tag="bc")
                    nc.tensor.matmul(
                        bps[:], bc2_sb[:], hrec[:], start=True, stop=True
                    )
                    rsb = hpool.tile([128, 512], F16, tag="rsb")
                    nc.vector.tensor_scalar_mul(rsb[:], bps[:], gscale_ap)
                    nc.vector.tensor_mul(qn[:, ssl], qps[:], rsb[:])

                def rotary_finish(qn, dst):
                    # rotate-half swap within each head (partition +-32)
                    qsw = cpool.tile([128, N], F16, tag="qsw", bufs=1)
                    for h0 in (0, 64):
                        nc.sync.dma_start(
                            out=qsw[h0 : h0 + 32, :], in_=qn[h0 + 32 : h0 + 64, :]
                        )
                        nc.sync.dma_start(
                            out=qsw[h0 + 32 : h0 + 64, :], in_=qn[h0 : h0 + 32, :]
                        )
                    nc.vector.tensor_mul(qn[:], qn[:], cos_sb[:])
                    nc.vector.tensor_mul(qsw[:], qsw[:], sin_sb[:])
                    nc.vector.tensor_add(dst[:], qn[:], qsw[:])

                def headnorm_rotary(proj_w, dst, gscale_ap):
                    qn = cpool.tile([128, N], F16, tag="qn", bufs=2, name="qn")
                    for s in range(4):
                        proj_norm_slice(proj_w, qn, s, gscale_ap)
                    rotary_finish(qn, dst)

                def v_tile(tb):
                    vps = ops_pool.tile([128, 2 * DH], F32, tag="olo", name="vps")
                    for c in range(NC):
                        nc.tensor.matmul(
                            vps[:],
                            xt_s[tb // 4][:, c, (tb % 4) * 128 : (tb % 4 + 1) * 128],
                            wv_sb[:, c, :],
                            start=(c == 0),
                            stop=(c == NC - 1),
                        )
                    nc.vector.tensor_copy(v_all[:, tb, 0:DH], vps[:, 0:DH])
                    nc.vector.tensor_copy(
                        v_all[:, tb, DH + 1 : 2 * DH + 1], vps[:, DH : 2 * DH]
                    )

                # ---- interleaved prologue: per strip, A-tiles then K/V/Q0
                qn_k = cpool.tile([128, N], F16, tag="qn", bufs=2, name="qn_k")
                qn_q0 = cpool.tile([128, N], F16, tag="qn", bufs=2, name="qn_q0")
                for s in range(4):
                    for tt in range(4 * s, 4 * s + 4):
                        a_tile(tt)
                    proj_norm_slice(lambda c: wk_sb[:, c, :], qn_k, s, gk_sb[:])
                    for tb in range(4 * s, 4 * s + 4):
                        v_tile(tb)
                    proj_norm_slice(
                        lambda c: wq_sb[:, c, 0:128], qn_q0, s, gq_sb[:, 0:1]
                    )
                rotary_finish(qn_k, kr_t)
                rotary_finish(qn_q0, qr_t[0])

                def attention(p):
                    for st in range(NSTRIP):
                        ssl = slice(st * STRIP, (st + 1) * STRIP)
                        o_lo = ops_pool.tile([DH + 1, STRIP], F32, tag="olo")
                        o_hi = ops_pool.tile([DH + 1, STRIP], F32, tag="ohi")

                        def s_step(c):
                            ck = slice(c * 128, (c + 1) * 128)
                            sps = sps_pool.tile(
                                [128, 2, STRIP], F32, tag="sps"
                            )
                            nc.tensor.matmul(
                                sps[:, 0, :],
                                kr_t[0:DH, ck],
                                qr_t[p][0:DH, ssl],
                                start=True,
                                stop=True,
                            )
                            nc.tensor.matmul(
                                sps[:, 1, :],
                                kr_t[DH:128, ck],
                                qr_t[p][DH:128, ssl],
                                start=True,
                                stop=True,
                            )
                            return sps

                        def exp_step(sps):
                            et = epool.tile([128, 2, STRIP], F16, tag="et")
                            nc.scalar.activation(
                                out=et[:],
                                in_=sps[:],
                                func=mybir.ActivationFunctionType.Exp,
                                bias=zero_t[:],
                                scale=float(DH) ** -0.5,
                            )
                            return et

                        def av_step(c, et):
                            nc.tensor.matmul(
                                o_lo[:],
                                v_all[:, c, 0 : DH + 1],
                                et[:, 0, :],
                                start=(c == 0),
                                stop=(c == NC - 1),
                            )
                            nc.tensor.matmul(
                                o_hi[:],
                                v_all[:, c, DH + 1 : 2 * DH + 2],
                                et[:, 1, :],
                                start=(c == 0),
                                stop=(c == NC - 1),
                            )

                        # S(c+1) ahead of AV(c): the PE queue never waits
                        # on exp(c)
                        et = exp_step(s_step(0))
                        for c in range(NC - 1):
                            sps = s_step(c + 1)
                            av_step(c, et)
                            et = exp_step(sps)
                        av_step(NC - 1, et)
                        # normalize rows 0..63 by row 64 (softmax sum);
                        # copy BOTH psum banks out first so they free before
                        # the recip/mul chain occupies the DVE queue
                        ocps = []
                        for ops in (o_lo, o_hi):
                            ocp = dpool.tile([DH + 1, STRIP], F16, tag="ocp")
                            nc.vector.tensor_copy(ocp[:], ops[:])
                            ocps.append(ocp)
                        for half in (0, 1):
                            ocp = ocps[half]
                            den = dpool.tile([1, STRIP], F16, tag="den")
                            with nc.allow_low_precision(reason="f16 den"):
                                nc.vector.reciprocal(
                                    out=den[:], in_=ocp[DH : DH + 1, :]
                                )
                            rsb2 = dpool.tile([DH, STRIP], F16, tag="rsb2")
                            nc.gpsimd.partition_broadcast(
                                rsb2[:], den[:], channels=DH
                            )
                            nc.vector.tensor_mul(
                                o_t[p][half * DH : (half + 1) * DH, ssl],
                                ocp[0:DH, :],
                                rsb2[:],
                            )

                # pipeline: headnorm(p+1) emitted before attention(p)
                for p in range(NPAIR):
                    if p + 1 < NPAIR:
                        headnorm_rotary(
                            lambda c, p=p: wq_sb[:, c, (p + 1) * 128 : (p + 2) * 128],
                            qr_t[p + 1],
                            gq_sb[:, p + 1 : p + 2],
                        )
                    attention(p)

        # ---------------- phase E: out-projection ----------------
        with ExitStack() as ee:
            wo_pool = ee.enter_context(tc.tile_pool(name="wop", bufs=1))
            outp = ee.enter_context(tc.tile_pool(name="outp", bufs=2))
            xps_pool = ee.enter_context(
                tc.tile_pool(name="xps", bufs=2, space="PSUM")
            )
            wo_sb = wo_pool.tile([128, NPAIR, DIM], F16)
            nc.sync.dma_start(
                out=wo_sb[:], in_=t_wo.rearrange("(p o) d -> o p d", o=128)
            )
            for tb in range(NT):
                xps = xps_pool.tile([128, DIM], F32, tag="xps")
                for p in range(NPAIR):
                    for s in range(4):
                        nc.tensor.matmul(
                            xps[:, s * 512 : (s + 1) * 512],
                            o_t[p][:, tb * 128 : (tb + 1) * 128],
                            wo_sb[:, p, s * 512 : (s + 1) * 512],
                            start=(p == 0),
                            stop=(p == NPAIR - 1),
                        )
                osb = outp.tile([128, DIM], F32, tag="osb")
                nc.vector.tensor_copy(osb[:], xps[:])
                nc.sync.dma_start(
                    out=t_out[tb * 128 : (tb + 1) * 128, :], in_=osb[:]
                )

    nc.compile()
    return nc


def _core_inputs(core, tokens, rotary, norm_w, Wq, Wkv, Wo, gamma_q, gamma_k):
    b, g = core // TPG, core % TPG
    # pair order: pair p = (8g+p, 8g+4+p); lo half -> kv head 2g, hi -> 2g+1
    heads = []
    for p in range(NPAIR):
        heads += [QH_PER * g + p, QH_PER * g + NPAIR + p]
    cols = np.concatenate([np.arange(h * DH, (h + 1) * DH) for h in heads])

    nw = norm_w[:, None].astype(np.float32)
    wq = (Wq * nw)[:, cols].astype(np.float16)
    kcols = np.arange(2 * g * DH, (2 * g + 2) * DH)
    wk = (Wkv[:, : KVH * DH] * nw)[:, kcols].astype(np.float16)
    wv = (Wkv[:, KVH * DH :] * nw)[:, kcols].astype(np.float16)
    wo = Wo[cols, :].astype(np.float16)

    gq = np.empty((NPAIR * 128, 1), np.float32)
    for p in range(NPAIR):
        gq[p * 128 : p * 128 + DH, 0] = (gamma_q[heads[2 * p]] + 1.0) * DH**0.5
        gq[p * 128 + DH : (p + 1) * 128, 0] = (
            gamma_q[heads[2 * p + 1]] + 1.0
        ) * DH**0.5
    gk = np.empty((128, 1), np.float32)
    gk[:DH, 0] = (gamma_k[2 * g] + 1.0) * DH**0.5
    gk[DH:, 0] = (gamma_k[2 * g + 1] + 1.0) * DH**0.5

    cosT = np.cos(rotary).T.astype(np.float32)  # [64, N]
    sinT = np.sin(rotary).T.astype(np.float32)
    sinS = np.concatenate([-sinT[:32], sinT[32:]], axis=0)
    cos_t = np.tile(cosT, (2, 1)).astype(np.float16)
    sins_t = np.tile(sinS, (2, 1)).astype(np.float16)

    sel2 = np.zeros((128, 2), np.float16)
    sel2[:DH, 0] = 1
    sel2[DH:, 1] = 1
    bc2 = np.zeros((2, 128), np.float32)
    bc2[0, :DH] = 1
    bc2[1, DH:] = 1

    return {
        "tokens": np.ascontiguousarray(tokens[b]),
        "wq": np.ascontiguousarray(wq),
        "wk": np.ascontiguousarray(wk),
        "wv": np.ascontiguousarray(wv),
        "wo": np.ascontiguousarray(wo),
        "gq": gq,
        "gk": gk,
        "cos_t": cos_t,
        "sins_t": sins_t,
        "sel2": sel2,
        "bc2": bc2,
        "ones_col": np.ones((1, DH), np.float32),
        "ident": np.eye(128, dtype=np.float16),
    }


def kernel(tokens, rotary, norm_w, Wq, Wkv, Wo, gamma_q, gamma_k, _bench=None):
    if "nc" not in _BUILD:
        _BUILD["nc"] = _build_nc()
    nc = _BUILD["nc"]

    in_maps = [
        _core_inputs(c, tokens, rotary, norm_w, Wq, Wkv, Wo, gamma_q, gamma_k)
        for c in range(NCORES)
    ]
    kw = dict(_bench or {})
    res = run_bass_kernel_spmd(nc, in_maps, list(range(NCORES)), **kw)
    if _bench is not None:
        _BUILD["last"] = res

    out = np.empty((B, N, DIM), np.float32)
    for b in range(B):
        acc = res.results[b * TPG]["out"].astype(np.float32)
        for g in range(1, TPG):
            acc = acc + res.results[b * TPG + g]["out"]
        out[b] = acc
    return out

